# revision 14
# baseline (speedup 1.0000x reference)
"""GQA multi-head attention (RoPE + tanh softcap + causal mask) on 8 TRN2 cores.

Sharding: tensor-parallel over the 8 kv-head groups (1 kv head + its 4 q heads
per core).  Each core computes its Q/K/V projections from the full hidden
states, runs attention for its 4 q heads, and produces a partial output
through its row-slice of Wo; the host sums the 8 partials.

v3 strategy (per core):
  - All-bf16 datapath (hs, Wq/Wk/Wv, cos/sin, q/k/v, softmax weights, Wo)
    with fp32 PSUM accumulation and fp32 softmax stats; fp16 output partials.
    Matmul throughput is unchanged (f32r was already 1 cycle/row) but DMA
    traffic halves, weight packets become 2-8KB runs, and SBUF fits the
    whole working set.
  - Projections run as two 3-bank PSUM sweeps (Q0,Q1,K then Q2,Q3,V) with
    the 32 hs d-chunk tiles held in SBUF across both sweeps.  This frees
    PSUM banks so attention/Wo matmuls can interleave with projections.
  - Software pipelining by emission order: block n's projection units are
    woven with block n-1's attention units and block n-2's Wo units, so the
    PE always has dense matmul work while the scalar engine runs the
    tanh/exp softmax chain.  Attention inner units are split into
    QK+softmax (U1/D1) and AV (U2/D2) stages with one-pair lookahead.
  - Softmax over the transposed [kcol, qrow] layout: tanh softcap bounds
    logits so no running max; denominator via ones-matmul over partitions,
    fast-reciprocal, gpsimd broadcast.  Diagonal chunks compute only the
    visible qrow suffix; the {0,1} mask multiply covers just the partial
    128-wide strip (deduplicated host-side).
  - DMA: weights/cos/sin/Wo/mask on the gpsimd queue (big contiguous runs,
    first pieces ordered so d=0 matmuls start ~13us in); hs tiles, RoPE
    rotate-half copies and fp16 output stores on the sync queue.
"""

import numpy as np

S, D, DH = 2048, 4096, 128
HQ, HKV = 32, 8
G = HQ // HKV            # q heads per core
N_CORES = 8
MULT = 0.08838834764831845
SOFTCAP = 30.0
ROPE_BASE = 10000.0
BLK = 512                # seq block (PSUM bank / matmul moving-dim limit)
NB = S // BLK            # 4 seq blocks
NCH = S // 128           # 16 kcol chunks
NDC = D // 128           # 32 contraction chunks for projections

_CACHE = {}


def _classify_mask(mask):
    """Per (qblock, kchunk) in the transposed [kcol, qrow-local] layout:
    skip (all masked), plain (all visible), or mixed.  Mixed chunks carry
    (lo, pe, off): visible qrow cols form the suffix [lo, 512); cols in
    [lo, pe) are partially masked (mask tile at offset `off` in the packed
    [128, total_width] bf16 mask tensor); cols [pe, 512) are fully visible.
    Identical mask tiles are deduplicated (causal masks repeat one strip).
    """
    m = np.asarray(mask).reshape(S, S)
    active = []
    mtiles = []
    offs = {}
    off = 0
    for n in range(NB):
        rows = m[n * BLK:(n + 1) * BLK]
        lst = []
        for c in range(NCH):
            sub = rows[:, c * 128:(c + 1) * 128]   # [qrow-local, kcol]
            vis = sub.any(axis=1)
            if not vis.any():
                continue
            full = sub.all(axis=1)
            if full.all():
                lst.append((c, 0, 0, -1))
                continue
            lo = int(np.argmax(vis))
            # visibility must be a suffix, and full-visibility a suffix of it
            assert vis[lo:].all(), "mask rows must be a suffix per chunk"
            if full[lo:].any():
                pe = lo + int(np.argmax(full[lo:]))
                assert full[pe:].all(), "full rows must form a suffix"
            else:
                pe = BLK
            tileT = np.ascontiguousarray(sub[lo:pe, :].T)  # [128, pe-lo]
            key = tileT.tobytes()
            if key not in offs:
                offs[key] = off
                mtiles.append(tileT)
                off += pe - lo
            lst.append((c, lo, pe, offs[key]))
        assert lst and lst[0][0] == 0 and lst[0][1] == 0, \
            "first active chunk must cover qrow col 0"
        active.append(tuple(lst))
    return tuple(active), mtiles


def _merge(xs, ys):
    """Distribute ys evenly among xs (both lists of thunks)."""
    if not ys:
        return list(xs)
    if not xs:
        return list(ys)
    out = []
    nx, ny = len(xs), len(ys)
    j = 0
    for i, x in enumerate(xs):
        out.append(x)
        while j * nx < (i + 1) * ny:
            out.append(ys[j])
            j += 1
    out.extend(ys[j:])
    return out


def _build(active, total_w):
    import concourse.bacc as bacc
    import concourse.mybir as mybir
    from concourse import tile
    from concourse.masks import make_identity
    from contextlib import ExitStack

    fp32 = mybir.dt.float32
    fp16 = mybir.dt.float16
    bf16 = mybir.dt.bfloat16
    AF = mybir.ActivationFunctionType

    nc = bacc.Bacc("TRN2", target_bir_lowering=False, debug=False,
                   enable_asserts=True, num_devices=N_CORES)
    hsT = nc.dram_tensor("hsT", [128, NDC, S], bf16, kind="ExternalInput").ap()
    wq = nc.dram_tensor("wq", [128, NDC, G * DH], bf16,
                        kind="ExternalInput").ap()
    wk = nc.dram_tensor("wk", [128, NDC, DH], bf16, kind="ExternalInput").ap()
    wv = nc.dram_tensor("wv", [128, NDC, DH], bf16, kind="ExternalInput").ap()
    wo = nc.dram_tensor("wo", [128, G, D], bf16, kind="ExternalInput").ap()
    cosT = nc.dram_tensor("cosT", [DH, S], bf16, kind="ExternalInput").ap()
    sinT = nc.dram_tensor("sinT", [DH, S], bf16, kind="ExternalInput").ap()
    maskm = (nc.dram_tensor("maskm", [128, total_w], bf16,
                            kind="ExternalInput").ap() if total_w else None)
    out = nc.dram_tensor("out", [S, D], fp16, kind="ExternalOutput").ap()

    with tile.TileContext(nc) as tc, ExitStack() as top:
        persist = top.enter_context(tc.tile_pool(name="persist", bufs=1))
        wq_sb = persist.tile([128, NDC, G * DH], bf16, tag="wq")
        wk_sb = persist.tile([128, NDC, DH], bf16, tag="wk")
        wv_sb = persist.tile([128, NDC, DH], bf16, tag="wv")
        wo_sb = persist.tile([128, G, D], bf16, tag="wo")
        cos_sb = persist.tile([DH, S], bf16, tag="cos")
        sin_sb = persist.tile([DH, S], bf16, tag="sin")
        identb = persist.tile([128, 128], bf16, tag="identb")
        ones_b = persist.tile([128, 1], bf16, tag="ones_b")
        mk_sb = (persist.tile([128, total_w], bf16, tag="mk", name="mk")
                 if total_w else None)
        qT = [[persist.tile([DH, BLK], bf16, tag=f"qT{h}_{n}",
                            name=f"qT{h}_{n}")
               for n in range(NB)] for h in range(G)]
        kT = [persist.tile([DH, BLK], bf16, tag=f"kT{n}", name=f"kT{n}")
              for n in range(NB)]
        vnat = [persist.tile([128, BLK], bf16, tag=f"vnat{n}",
                             name=f"vnat{n}") for n in range(NB)]
        attnT = [[persist.tile([DH, BLK], bf16, tag=f"attnT{h}_{n}",
                               name=f"attnT{h}_{n}")
                  for n in range(NB)] for h in range(G)]

        # ---- working pools ------------------------------------------------
        hsp = top.enter_context(tc.tile_pool(name="hs", bufs=9))
        rawp = top.enter_context(tc.tile_pool(name="raw", bufs=2))
        rotp = top.enter_context(tc.tile_pool(name="rot", bufs=2))
        tmpp = top.enter_context(tc.tile_pool(name="tmp", bufs=2))
        vtp = top.enter_context(tc.tile_pool(name="vtp", bufs=1))
        ttp = top.enter_context(tc.tile_pool(name="ttp", bufs=2))
        ttsp = top.enter_context(tc.tile_pool(name="ttsp", bufs=2))
        wtp = top.enter_context(tc.tile_pool(name="wtp", bufs=2))
        wtsp = top.enter_context(tc.tile_pool(name="wtsp", bufs=2))
        wsp = top.enter_context(tc.tile_pool(name="wsp", bufs=2))
        dsp = top.enter_context(tc.tile_pool(name="dsp", bufs=2))
        bcp = top.enter_context(tc.tile_pool(name="bcp", bufs=1))
        osb = top.enter_context(tc.tile_pool(name="osb", bufs=3))
        # PSUM: 3 (proj sweep) + 3 (qk+dn) + 1 (av) + 1 (wo pso / v-transpose)
        projps = top.enter_context(
            tc.tile_pool(name="projps", bufs=3, space="PSUM"))
        qkps = top.enter_context(
            tc.tile_pool(name="qkps", bufs=2, space="PSUM"))
        avps = top.enter_context(
            tc.tile_pool(name="avps", bufs=1, space="PSUM"))
        wops = top.enter_context(
            tc.tile_pool(name="wops", bufs=2, space="PSUM"))

        # ---- constant / weight DMAs (gpsimd queue, startup-ordered) -------
        make_identity(nc, identb[:])
        nc.vector.memset(ones_b[:], 1.0)
        # first pieces small so the d=0 matmuls start as early as possible;
        # wv last in each trio (V matmuls run in sweep2, ~25us later)
        for i in range(2):
            ks = slice(2 * i, 2 * i + 2)
            nc.gpsimd.dma_start(wk_sb[:, ks, :], wk[:, ks, :])
            nc.gpsimd.dma_start(wq_sb[:, ks, :], wq[:, ks, :])
        nc.gpsimd.dma_start(wv_sb[:, 0:4, :], wv[:, 0:4, :])
        for i in range(1, 8):
            ks = slice(4 * i, 4 * i + 4)
            nc.gpsimd.dma_start(wk_sb[:, ks, :], wk[:, ks, :])
            nc.gpsimd.dma_start(wq_sb[:, ks, :], wq[:, ks, :])
            nc.gpsimd.dma_start(wv_sb[:, ks, :], wv[:, ks, :])
        nc.gpsimd.dma_start(cos_sb[:], cosT[:])
        nc.gpsimd.dma_start(sin_sb[:], sinT[:])
        if total_w:
            nc.gpsimd.dma_start(mk_sb[:], maskm[:])

        evict_flip = [0]

        def evict(dst, src):
            if evict_flip[0] % 2 == 0:
                nc.scalar.copy(dst, src)
            else:
                nc.vector.tensor_copy(dst, src)
            evict_flip[0] += 1

        hs_tiles = {}

        # ---- projection units --------------------------------------------
        HSG = 4              # d-chunks per hs DMA piece

        def proj_units(n):
            """Units for block n.  Returns (sweep1, mid, sweep2, btail):
            sweep1 = Q0,Q1,K d-loop (hs pieces all issued at start);
            mid    = rope(Q0), rope(Q1), rope(K);
            sweep2 = Q2,Q3,V d-loop;
            btail  = rope(Q2), rope(Q3), V-transpose  (emit in next slot).
            """
            sl = slice(n * BLK, (n + 1) * BLK)
            st = {}

            def rope(ps_key, idx, dest):
                def u():
                    ps = st[ps_key][idx]
                    raw = rawp.tile([128, BLK], bf16, tag="raw")
                    evict(raw[:], ps[:])
                    rot = rotp.tile([128, BLK], bf16, tag="rot")
                    nc.sync.dma_start(rot[0:64, :], raw[64:128, :])
                    nc.sync.dma_start(rot[64:128, :], raw[0:64, :])
                    tmp = tmpp.tile([128, BLK], bf16, tag="tmp")
                    nc.vector.tensor_mul(tmp[:], raw[:], cos_sb[:, sl])
                    nc.vector.tensor_mul(rot[:], rot[:], sin_sb[:, sl])
                    nc.vector.tensor_add(dest[:], tmp[:], rot[:])
                return u

            def s1(d):
                def u():
                    if d == 0:
                        st['s1'] = [projps.tile([128, BLK], fp32, tag="pps",
                                                name="pps")
                                    for _ in range(3)]
                        for p in range(NDC // HSG):
                            hs_t = hsp.tile([128, HSG, BLK], bf16, tag="hs",
                                            name="hs")
                            nc.sync.dma_start(
                                hs_t[:], hsT[:, p * HSG:(p + 1) * HSG, sl])
                            hs_tiles[(n, p)] = hs_t
                    hs_t = hs_tiles[(n, d // HSG)][:, d % HSG, :]
                    ps = st['s1']
                    fl = dict(start=(d == 0), stop=(d == NDC - 1),
                              skip_group_check=True)
                    nc.tensor.matmul(ps[0][:], wq_sb[:, d, 0:DH],
                                     hs_t, **fl)
                    nc.tensor.matmul(ps[1][:], wq_sb[:, d, DH:2 * DH],
                                     hs_t, **fl)
                    nc.tensor.matmul(ps[2][:], wk_sb[:, d, :], hs_t, **fl)
                return u

            def s2(d):
                def u():
                    if d == 0:
                        st['s2'] = [projps.tile([128, BLK], fp32, tag="pps",
                                                name="pps")
                                    for _ in range(3)]
                    piece = hs_tiles[(n, d // HSG)]
                    hs_t = piece[:, d % HSG, :]
                    if d % HSG == HSG - 1:
                        del hs_tiles[(n, d // HSG)]
                    ps = st['s2']
                    fl = dict(start=(d == 0), stop=(d == NDC - 1),
                              skip_group_check=True)
                    nc.tensor.matmul(ps[0][:], wq_sb[:, d, 2 * DH:3 * DH],
                                     hs_t, **fl)
                    nc.tensor.matmul(ps[1][:], wq_sb[:, d, 3 * DH:4 * DH],
                                     hs_t, **fl)
                    nc.tensor.matmul(ps[2][:], wv_sb[:, d, :], hs_t, **fl)
                return u

            def vproj():
                def u():
                    vt = vtp.tile([128, BLK], bf16, tag="vt")
                    nc.scalar.copy(vt[:], st['s2'][2][:])
                    tp = wops.tile([128, BLK], bf16, tag="pso")
                    for j in range(BLK // 128):
                        nc.tensor.matmul(tp[:, j * 128:(j + 1) * 128],
                                         vt[:, j * 128:(j + 1) * 128],
                                         identb[:], is_transpose=True,
                                         start=True, stop=True,
                                         skip_group_check=True)
                    nc.vector.tensor_copy(vnat[n][:], tp[:])
                return u

            sweep1 = [s1(d) for d in range(NDC)]
            mid = [rope('s1', 0, qT[0][n]), rope('s1', 1, qT[1][n]),
                   rope('s1', 2, kT[n])]
            sweep2 = [s2(d) for d in range(NDC)]
            btail = [rope('s2', 0, qT[2][n]), rope('s2', 1, qT[3][n]),
                     vproj()]
            return sweep1, mid, sweep2, btail

        # ---- attention units ---------------------------------------------
        def attn_units(n):
            acts = active[n]
            plains = [c for (c, lo, pe, off) in acts if pe == 0]
            diags = [(c, lo, pe, off) for (c, lo, pe, off) in acts
                     if pe != 0]
            assert len(plains) % 2 == 0 and len(diags) <= 4
            pairs = [(plains[i], plains[i + 1])
                     for i in range(0, len(plains), 2)]
            n_ch = len(acts)

            def head_units(h):
                st = {}

                def begin():
                    st['av'] = avps.tile([128, BLK], fp32, tag="av", name="av")
                    st['ws'] = wsp.tile([128, 2 * BLK], bf16, tag="ws",
                                        name="ws")
                    st['first'] = [True, True]
                    st['vstart'] = [0, 0]
                    st['mm'] = 0

                def u1(c0, c1, first):
                    def u():
                        if first:
                            begin()
                        tt = ttp.tile([128, 2 * BLK], fp32, tag="tt")
                        for i, c in enumerate((c0, c1)):
                            qk = qkps.tile([128, BLK], fp32, tag="qk")
                            nc.tensor.matmul(
                                qk[:],
                                kT[c // 4][:, (c % 4) * 128:(c % 4 + 1) * 128],
                                qT[h][n][:], start=True, stop=True)
                            nc.scalar.activation(
                                tt[:, i * BLK:(i + 1) * BLK], qk[:],
                                AF.Tanh, scale=1.0 / SOFTCAP)
                        wt = wtp.tile([128, 2 * BLK], bf16, tag="wt")
                        nc.scalar.activation(wt[:], tt[:], AF.Exp,
                                             scale=SOFTCAP)
                        ws = st['ws']
                        if st['first'][0]:
                            nc.vector.tensor_copy(ws[:], wt[:])
                            st['first'] = [False, False]
                        else:
                            nc.vector.tensor_add(ws[:], ws[:], wt[:])
                        st[('wt', c0)] = wt
                    return u

                def u2(c0, c1):
                    def u():
                        wt = st.pop(('wt', c0))
                        for i, c in enumerate((c0, c1)):
                            nc.tensor.matmul(
                                st['av'][:],
                                vnat[c // 4][:, (c % 4) * 128:
                                             (c % 4 + 1) * 128],
                                wt[:, i * BLK:(i + 1) * BLK],
                                start=(st['mm'] == 0),
                                stop=(st['mm'] == n_ch - 1),
                                skip_group_check=True)
                            st['mm'] += 1
                    return u

                def d1(c, lo, pe, off, first):
                    def u():
                        if first:
                            begin()
                        qk = qkps.tile([128, BLK], fp32, tag="qk")
                        nc.tensor.matmul(
                            qk[:, lo:],
                            kT[c // 4][:, (c % 4) * 128:(c % 4 + 1) * 128],
                            qT[h][n][:, lo:], start=True, stop=True)
                        tts_t = ttsp.tile([128, BLK], fp32, tag="tts")
                        nc.scalar.activation(tts_t[:, lo:], qk[:, lo:],
                                             AF.Tanh, scale=1.0 / SOFTCAP)
                        wts_t = wtsp.tile([128, BLK], bf16, tag="wts")
                        nc.scalar.activation(wts_t[:, lo:], tts_t[:, lo:],
                                             AF.Exp, scale=SOFTCAP)
                        if pe > lo:
                            nc.vector.tensor_mul(
                                wts_t[:, lo:pe], wts_t[:, lo:pe],
                                mk_sb[:, off:off + pe - lo])
                        p = c % 2
                        ws = st['ws']
                        dst = ws[:, p * BLK + lo:(p + 1) * BLK]
                        if st['first'][p]:
                            nc.vector.tensor_copy(dst, wts_t[:, lo:])
                            st['first'][p] = False
                            st['vstart'][p] = lo
                        else:
                            nc.vector.tensor_add(dst, dst, wts_t[:, lo:])
                        st[('wts', c)] = wts_t
                    return u

                def d2(c, lo):
                    def u():
                        wts_t = st.pop(('wts', c))
                        nc.tensor.matmul(
                            st['av'][:, lo:],
                            vnat[c // 4][:, (c % 4) * 128:(c % 4 + 1) * 128],
                            wts_t[:, lo:],
                            start=(st['mm'] == 0),
                            stop=(st['mm'] == n_ch - 1),
                            skip_group_check=True)
                        st['mm'] += 1
                    return u

                def fin():
                    def u():
                        dn = qkps.tile([1, BLK], fp32, tag="qk")
                        ws = st['ws']
                        for p in (0, 1):
                            vs = st['vstart'][p]
                            nc.tensor.matmul(
                                dn[:, vs:], ones_b[:],
                                ws[:, p * BLK + vs:(p + 1) * BLK],
                                start=(p == 0), stop=(p == 1),
                                skip_group_check=True)
                        dns = dsp.tile([1, BLK], fp32, tag="dns")
                        nc.vector.reciprocal_approx_fast(dns[:], dn[:])
                        bc = bcp.tile([128, BLK], fp32, tag="bc")
                        nc.gpsimd.partition_broadcast(bc[:], dns[:])
                        nc.vector.tensor_mul(attnT[h][n][:], st['av'][:],
                                             bc[:])
                    return u

                # stage-split emission with one-unit lookahead:
                # early = pairs (U1(p0) U1(p1) U2(p0) ... U2(p_last));
                # late  = diags (D1(c0) D1(c1) D2(c0) ... D2(c_last)) + FIN
                def weave(s1s, s2s):
                    if not s1s:
                        return []
                    o = [s1s[0]]
                    for i in range(1, len(s1s)):
                        o.append(s1s[i])
                        o.append(s2s[i - 1])
                    o.append(s2s[-1])
                    return o

                first = True
                p1 = []
                for (c0, c1) in pairs:
                    p1.append(u1(c0, c1, first))
                    first = False
                p2 = [u2(c0, c1) for (c0, c1) in pairs]
                q1 = []
                for (c, lo, pe, off) in diags:
                    q1.append(d1(c, lo, pe, off, first))
                    first = False
                q2 = [d2(c, lo) for (c, lo, pe, off) in diags]
                early = weave(p1, p2)
                late = weave(q1, q2) + [fin()]
                return early, late

            return [head_units(h) for h in range(G)]

        # ---- Wo units -----------------------------------------------------
        def wo_units(n2):
            units = []
            st = {}
            for j in range(BLK // 128):
                s = n2 * (BLK // 128) + j
                for nn in range(D // BLK):
                    def u(s=s, j=j, nn=nn):
                        pso = wops.tile([128, BLK], fp32, tag="pso")
                        for h2 in range(G):
                            nc.tensor.matmul(
                                pso[:],
                                attnT[h2][n2][:, j * 128:(j + 1) * 128],
                                wo_sb[:, h2, nn * BLK:(nn + 1) * BLK],
                                start=(h2 == 0), stop=(h2 == G - 1),
                                skip_group_check=True)
                        if nn % 2 == 0:
                            st['ot'] = osb.tile([128, 2 * BLK], fp16,
                                                tag="ot", name="ot")
                        half = nn % 2
                        nc.vector.tensor_copy(
                            st['ot'][:, half * BLK:(half + 1) * BLK], pso[:])
                        if half == 1:
                            nc.sync.dma_start(
                                out[s * 128:(s + 1) * 128,
                                    (nn - 1) * BLK:(nn + 1) * BLK],
                                st['ot'][:])
                    units.append(u)
            return units

        # ---- schedule -----------------------------------------------------
        # Slot n: sweep1(n) woven with [btail(n-1), h0-late(n-1), h1(n-1),
        # h2(n-1)]; then mid ropes; then sweep2(n) woven with [h3(n-1),
        # h0-early(n)].  wo(n-2) units are spread across both halves.
        # Tail: btail(3) + h0-late(3) + h1..h3(3) woven with wo(2); wo(3).
        # Heads open their av PSUM strictly sequentially in emission order
        # (avps bufs=1), so av-slot waits can never deadlock the PE.
        def wo_load(g):
            def u():
                gs = slice(g * 1024, (g + 1) * 1024)
                nc.gpsimd.dma_start(wo_sb[:, :, gs], wo[:, :, gs])
            return u

        prev_btail = []
        prev_heads = None      # [(early, late)] of previous block
        pend_h0_late = []
        for n in range(NB):
            s1u, midu, s2u, btail = proj_units(n)
            heads = attn_units(n)
            if n == 0:
                # wo weight loads overlap b0's sweep2 (hs(b0) is already
                # resident, so the DMA engines are free); done before the
                # hs(b1) burst at slot-1 start.
                midu = midu + [wo_load(g) for g in range(4)]
            ya = list(pend_h0_late)
            yb = []
            w = []
            if n >= 1:
                ya += prev_heads[1][0] + prev_heads[1][1]
                ya += prev_heads[2][0] + prev_heads[2][1]
                yb += prev_heads[3][0] + prev_heads[3][1]
            if n >= 2:
                w += wo_units(n - 2)
            yb += heads[0][0]
            pend_h0_late = heads[0][1]
            wa = w[:len(w) // 2]
            wb = w[len(w) // 2:]
            for u in _merge(s1u, _merge(prev_btail + ya, wa)):
                u()
            for u in midu:
                u()
            for u in _merge(s2u, _merge(yb, wb)):
                u()
            prev_btail = btail
            prev_heads = heads
        ys = prev_btail + pend_h0_late
        for h in (1, 2, 3):
            ys += prev_heads[h][0] + prev_heads[h][1]
        # hold back the last 8 wo(b2) units so the PE has dense work while
        # the final head's softmax-denominator chain completes
        w2 = wo_units(NB - 2)
        for u in _merge(ys, w2[:-8]):
            u()
        for u in w2[-8:]:
            u()
        for u in wo_units(NB - 1):
            u()

    nc.compile()
    return nc


def _rope_tables():
    import ml_dtypes
    bf16 = ml_dtypes.bfloat16
    j = np.arange(0, DH, 2, dtype=np.float32)
    inv = np.float32(1.0) / (np.float32(ROPE_BASE) ** (j / np.float32(DH)))
    t = np.arange(S, dtype=np.float32)
    phase = t[:, None] * inv[None, :]          # [S, 64] fp32 like reference
    cos = np.cos(phase).astype(np.float32)     # [S, 64]
    sin = np.sin(phase).astype(np.float32)
    cosT = np.concatenate([cos.T, cos.T], axis=0)              # [128, S]
    sinT = np.concatenate([-sin.T, sin.T], axis=0)             # sign-folded
    return (np.ascontiguousarray(cosT).astype(bf16),
            np.ascontiguousarray(sinT).astype(bf16))


def _in_maps(hidden_states, mask, Wq, Wk, Wv, Wo):
    import ml_dtypes
    bf16 = ml_dtypes.bfloat16

    hs = np.asarray(hidden_states, dtype=np.float32).reshape(S, D)
    Wq = np.asarray(Wq, dtype=np.float32)
    Wk = np.asarray(Wk, dtype=np.float32)
    Wv = np.asarray(Wv, dtype=np.float32)
    Wo = np.asarray(Wo, dtype=np.float32)
    active, mtiles = _classify_mask(mask)
    mt = (np.concatenate(mtiles, axis=1).astype(bf16)
          if mtiles else None)
    # hsT packed [128, NDC, S]: partition-major d-chunks, contiguous seq runs
    hsT = np.ascontiguousarray(
        hs.T.reshape(NDC, 128, S).transpose(1, 0, 2)).astype(bf16)
    cosT, sinT = _rope_tables()

    def pack_w(w, width):
        # [D, width] -> [128, NDC, width] partition-major
        return np.ascontiguousarray(
            w.reshape(NDC, 128, width).transpose(1, 0, 2)).astype(bf16)

    maps = []
    for c in range(N_CORES):
        m = {
            "hsT": hsT,
            "wq": pack_w(Wq[:, c * G * DH:(c + 1) * G * DH]
                         * np.float32(MULT), G * DH),
            "wk": pack_w(Wk[:, c * DH:(c + 1) * DH], DH),
            "wv": pack_w(Wv[:, c * DH:(c + 1) * DH], DH),
            "wo": np.ascontiguousarray(
                Wo[c * G * DH:(c + 1) * G * DH, :].reshape(G, 128, D)
                .transpose(1, 0, 2)).astype(bf16),
            "cosT": cosT,
            "sinT": sinT,
        }
        if mt is not None:
            m["maskm"] = np.ascontiguousarray(mt)
        maps.append(m)
    return active, mt, maps


def kernel(hidden_states, mask, Wq, Wk, Wv, Wo):
    from concourse.bass_utils import run_bass_kernel_spmd

    active, mt, maps = _in_maps(hidden_states, mask, Wq, Wk, Wv, Wo)
    key = active
    if key not in _CACHE:
        _CACHE[key] = _build(active, 0 if mt is None else mt.shape[1])
    nc = _CACHE[key]

    res = run_bass_kernel_spmd(nc, maps, list(range(N_CORES)))
    acc = np.zeros((S, D), dtype=np.float32)
    for c in range(N_CORES):
        acc += res.results[c]["out"].astype(np.float32)
    return acc.reshape(1, S, D)
